# revision 16
# baseline (speedup 1.0000x reference)
"""Trainium2 Bass kernel for CoarseDirectionReducer (segment_reduce).

out[b, n, g, :, :] = sum_j softmax(logits)[g, j] * x[b, n, GROUP_IDX[g, j], :, :]

Sharding: pure data parallel. The 16 (b, n) slices are split 2-per-core
across 8 NeuronCores; the tiny (8,3) softmax weights are computed on host
and baked into the kernel as immediates.

Per-core layout: each 256x256 channel plane is viewed as (128 partitions,
512), split into 2 half-plane chunks of 256 columns. One HWDGE DMA loads
all 24 channels of a chunk as a (128, 24, 256) SBUF tile (3 MB, 1 KB
contiguous segments per partition); each output group is one ACT
scaled-copy plus two fused DVE (x*w)+acc ops; one DMA stores (128, 8, 256).
"""

import numpy as np

import concourse.bass as bass
import concourse.mybir as mybir
from concourse.bass_utils import run_bass_kernel_spmd
from concourse.tile import TileContext
from concourse.vector_clock import ScopedClock, VectorClock


class SingleWaitTileContext(TileContext):
    """TileContext whose kernel-tail drain never carries more than one
    embedded sync wait.

    The walrus build in this container rejects instructions with more than
    one sync wait command. Tile's tail drain waits on every outstanding
    proc sem at once; split those into a chain of single-wait nops on the
    drain engine first, so the real drain has nothing left to wait on.
    """

    def _drain_and_barrier(self, tick_clock, wait_clock):
        gc = tick_clock.global_clock
        for proc in range(len(gc)):
            tick = gc[proc]
            if tick <= 0:
                continue
            nop = self.nc.sync.nop(nofuse=True, hint="drain_split")
            vc = VectorClock()
            vc.require_at_least(proc, tick)
            wait_clock.add_sem_waits(nop.ins, ScopedClock({None: vc}))
        # Same as TileContext._drain_and_barrier, but with no sem waits on
        # the drain itself: the nop chain above already made SP wait for
        # every outstanding proc, and the drain follows them in SP program
        # order. Tail sem clears + second barrier are dropped — the kernel
        # preamble clears semaphores on every execution anyway.
        self.nc.sync.drain()
        self.nc.all_engine_barrier()
        assert self.sems is not None
        popped = self.nc._tile_sem_poison_stack.pop()
        assert popped is self._sem_poison

N_CORES = 8
B, NCOARSE, NUM_FINE, H, W = 4, 4, 24, 256, 256
NGROUPS = 3  # members per group
NOUT = 8  # direction groups
SLICES = B * NCOARSE  # 16 (b, n) slices
SLICES_PER_CORE = SLICES // N_CORES  # 2
P = 128  # SBUF partitions; one plane = (128, 512)
PLANE_F = (H * W) // P  # 512
NH = 2  # half-plane chunks
F = PLANE_F // NH  # 256

GROUPS_DXDY = (((1, 1), (2, 2), (2, 1)), ((0, 1), (0, 2), (1, 2)),
               ((-1, 1), (-2, 2), (-1, 2)), ((1, 0), (2, 0), (2, -1)),
               ((-1, 0), (-2, 0), (-2, 1)), ((1, -1), (2, -2), (1, -2)),
               ((0, -1), (0, -2), (-1, -2)), ((-1, -1), (-2, -2), (-2, -1)))


def _group_indices():
    offsets_dydx = [(dy, dx) for dy in range(-2, 3) for dx in range(-2, 3)
                    if (dy, dx) != (0, 0)]
    off_to_idx = {(dx, dy): i for i, (dy, dx) in enumerate(offsets_dydx)}
    return np.array([[off_to_idx[o] for o in g] for g in GROUPS_DXDY],
                    dtype=np.int32)  # (8, 3)


GROUP_IDX = _group_indices()

_LAST_RESULT = None  # BassKernelResults of the most recent run (for test.py)


def build_nc(wg: np.ndarray) -> bass.Bass:
    """Build the per-core Bass program. wg: (8, 3) f32 softmax weights."""
    f32 = mybir.dt.float32
    mult = mybir.AluOpType.mult
    add = mybir.AluOpType.add
    copy_fn = mybir.ActivationFunctionType.Copy

    nc = bass.Bass()
    x = nc.declare_dram_parameter(
        "x", [SLICES_PER_CORE, NUM_FINE, P, PLANE_F], f32, isOutput=False)
    y = nc.declare_dram_parameter(
        "y", [SLICES_PER_CORE, NOUT, P, PLANE_F], f32, isOutput=True)

    # Uneven chunks per slice: big chunk first to fill the pipe, smaller
    # chunk last so the post-stream tail (compute+store of the final
    # chunk) is short. 4 chunks total keeps each HWDGE ring under the
    # queue-depth threshold that would add a second (illegal) sync wait.
    chunks = [(0, 320), (320, 192)]

    with SingleWaitTileContext(nc) as tc:
        with (
            tc.tile_pool(name="xin_a", bufs=SLICES_PER_CORE) as xin_a,
            tc.tile_pool(name="xin_b", bufs=SLICES_PER_CORE) as xin_b,
            tc.tile_pool(name="yout_a", bufs=SLICES_PER_CORE) as yout_a,
            tc.tile_pool(name="yout_b", bufs=SLICES_PER_CORE) as yout_b,
        ):
            # Per group, sort members by ascending weight so the two Horner
            # ratios are <= 1 and the final scale is the largest weight.
            order = [sorted(range(NGROUPS), key=lambda j: wg[g][j])
                     for g in range(NOUT)]
            for s in range(SLICES_PER_CORE):
                for off, fw in chunks:
                    xin_pool = xin_a if fw == 320 else xin_b
                    yout_pool = yout_a if fw == 320 else yout_b
                    it = xin_pool.tile([P, NUM_FINE, fw], f32)
                    nc.sync.dma_start(
                        it[:], x[s, :, :, off:off + fw]
                        .rearrange("c p f -> p c f"))
                    ot = yout_pool.tile([P, NOUT, fw], f32)
                    for g in range(NOUT):
                        js, jm, jl = order[g]
                        i0, i1, i2 = (int(GROUP_IDX[g][j])
                                      for j in (js, jm, jl))
                        ws, wm, wl = (float(wg[g][j]) for j in (js, jm, jl))
                        # t = (x_s*(ws/wm) + x_m); t = t*(wm/wl) + x_l;
                        # out = t*wl  — two fused DVE ops + ACT scale.
                        # All 8 in-place ACT scales supersede the DVE
                        # writes region-by-region, so the out-DMA needs
                        # only the single ACT sem wait (1-wait ISA limit).
                        nc.vector.scalar_tensor_tensor(
                            ot[:, g], it[:, i0], ws / wm, it[:, i1],
                            mult, add)
                        nc.vector.scalar_tensor_tensor(
                            ot[:, g], ot[:, g], wm / wl, it[:, i2],
                            mult, add)
                        nc.scalar.activation(
                            ot[:, g], ot[:, g], copy_fn, scale=wl)
                    nc.scalar.dma_start(
                        y[s, :, :, off:off + fw].rearrange("g p f -> p g f"),
                        ot[:])
    return nc


def _softmax_rows(logits: np.ndarray) -> np.ndarray:
    z = logits.astype(np.float32)
    z = z - z.max(axis=-1, keepdims=True)
    e = np.exp(z)
    return e / e.sum(axis=-1, keepdims=True)


def kernel(fine_features: np.ndarray, logits: np.ndarray) -> np.ndarray:
    global _LAST_RESULT
    ff = np.asarray(fine_features, dtype=np.float32)
    wg = _softmax_rows(np.asarray(logits, dtype=np.float32))

    # (B, 96, H, W) -> (16 slices, 24, 128, 512); slicing the outer axis
    # keeps each core's shard a contiguous zero-copy view.
    x16 = ff.reshape(SLICES, NUM_FINE, P, PLANE_F)
    in_maps = [
        {"x": x16[SLICES_PER_CORE * k:SLICES_PER_CORE * (k + 1)]}
        for k in range(N_CORES)
    ]

    nc = build_nc(wg)
    res = run_bass_kernel_spmd(nc, in_maps, core_ids=list(range(N_CORES)))
    _LAST_RESULT = res

    out16 = np.concatenate([res.results[k]["y"] for k in range(N_CORES)],
                           axis=0)  # (16, 8, 128, 512)
    return out16.reshape(B, NCOARSE * NOUT, H, W)


# revision 19
# speedup vs baseline: 1.1500x; 1.1500x over previous
"""Trainium2 Bass kernel for CoarseDirectionReducer (segment_reduce).

out[b, n, g, :, :] = sum_j softmax(logits)[g, j] * x[b, n, GROUP_IDX[g, j], :, :]

Sharding: pure data parallel. The 16 (b, n) slices are split 2-per-core
across 8 NeuronCores; the tiny (8,3) softmax weights are computed on host
and baked into the kernel as immediates.

Per-core layout: each 256x256 channel plane is viewed as (128 partitions,
512), split into 2 half-plane chunks of 256 columns. One HWDGE DMA loads
all 24 channels of a chunk as a (128, 24, 256) SBUF tile (3 MB, 1 KB
contiguous segments per partition); each output group is one ACT
scaled-copy plus two fused DVE (x*w)+acc ops; one DMA stores (128, 8, 256).
"""

import numpy as np

import concourse.bass as bass
import concourse.mybir as mybir
from concourse.bass_utils import run_bass_kernel_spmd
from concourse.tile import TileContext
from concourse.vector_clock import ScopedClock, VectorClock


class SingleWaitTileContext(TileContext):
    """TileContext whose kernel-tail drain never carries more than one
    embedded sync wait.

    The walrus build in this container rejects instructions with more than
    one sync wait command. Tile's tail drain waits on every outstanding
    proc sem at once; split those into a chain of single-wait nops on the
    drain engine first, so the real drain has nothing left to wait on.
    """

    def _drain_and_barrier(self, tick_clock, wait_clock):
        gc = tick_clock.global_clock
        for proc in range(len(gc)):
            tick = gc[proc]
            if tick <= 0:
                continue
            nop = self.nc.sync.nop(nofuse=True, hint="drain_split")
            vc = VectorClock()
            vc.require_at_least(proc, tick)
            wait_clock.add_sem_waits(nop.ins, ScopedClock({None: vc}))
        # Same as TileContext._drain_and_barrier, but with no sem waits on
        # the drain itself: the nop chain above already made SP wait for
        # every outstanding proc, and the drain follows them in SP program
        # order. Tail sem clears + second barrier are dropped — the kernel
        # preamble clears semaphores on every execution anyway.
        self.nc.sync.drain()
        self.nc.all_engine_barrier()
        assert self.sems is not None
        popped = self.nc._tile_sem_poison_stack.pop()
        assert popped is self._sem_poison

N_CORES = 8
B, NCOARSE, NUM_FINE, H, W = 4, 4, 24, 256, 256
NGROUPS = 3  # members per group
NOUT = 8  # direction groups
SLICES = B * NCOARSE  # 16 (b, n) slices
SLICES_PER_CORE = SLICES // N_CORES  # 2
P = 128  # SBUF partitions; one plane = (128, 512)
PLANE_F = (H * W) // P  # 512
NH = 2  # half-plane chunks
F = PLANE_F // NH  # 256

GROUPS_DXDY = (((1, 1), (2, 2), (2, 1)), ((0, 1), (0, 2), (1, 2)),
               ((-1, 1), (-2, 2), (-1, 2)), ((1, 0), (2, 0), (2, -1)),
               ((-1, 0), (-2, 0), (-2, 1)), ((1, -1), (2, -2), (1, -2)),
               ((0, -1), (0, -2), (-1, -2)), ((-1, -1), (-2, -2), (-2, -1)))


def _group_indices():
    offsets_dydx = [(dy, dx) for dy in range(-2, 3) for dx in range(-2, 3)
                    if (dy, dx) != (0, 0)]
    off_to_idx = {(dx, dy): i for i, (dy, dx) in enumerate(offsets_dydx)}
    return np.array([[off_to_idx[o] for o in g] for g in GROUPS_DXDY],
                    dtype=np.int32)  # (8, 3)


GROUP_IDX = _group_indices()

_LAST_RESULT = None  # BassKernelResults of the most recent run (for test.py)


def build_nc(wg: np.ndarray) -> bass.Bass:
    """Build the per-core Bass program. wg: (8, 3) f32 softmax weights."""
    f32 = mybir.dt.float32
    mult = mybir.AluOpType.mult
    add = mybir.AluOpType.add
    copy_fn = mybir.ActivationFunctionType.Copy

    nc = bass.Bass()
    x = nc.declare_dram_parameter(
        "x", [SLICES_PER_CORE, NUM_FINE, P, PLANE_F], f32, isOutput=False)
    y = nc.declare_dram_parameter(
        "y", [SLICES_PER_CORE, NOUT, P, PLANE_F], f32, isOutput=True)

    # Two half-plane chunks per slice. 4 chunks total keeps each HWDGE
    # ring under the queue-depth threshold that would add a second
    # (illegal) sync wait.
    chunks = [(0, 256), (256, 256)]

    with SingleWaitTileContext(nc) as tc:
        with (
            tc.tile_pool(name="xin_a", bufs=SLICES_PER_CORE) as xin_a,
            tc.tile_pool(name="xin_b", bufs=SLICES_PER_CORE) as xin_b,
            tc.tile_pool(name="yout_a", bufs=SLICES_PER_CORE) as yout_a,
            tc.tile_pool(name="yout_b", bufs=SLICES_PER_CORE) as yout_b,
        ):
            # Per group, sort members by ascending weight so the two Horner
            # ratios are <= 1 and the final scale is the largest weight.
            order = [sorted(range(NGROUPS), key=lambda j: wg[g][j])
                     for g in range(NOUT)]
            for s in range(SLICES_PER_CORE):
                for off, fw in chunks:
                    xin_pool = xin_a if off == 0 else xin_b
                    yout_pool = yout_a if off == 0 else yout_b
                    it = xin_pool.tile([P, NUM_FINE, fw], f32)
                    nc.sync.dma_start(
                        it[:], x[s, :, :, off:off + fw]
                        .rearrange("c p f -> p c f"))
                    ot = yout_pool.tile([P, NOUT, fw], f32)
                    for g in range(NOUT):
                        js, jm, jl = order[g]
                        i0, i1, i2 = (int(GROUP_IDX[g][j])
                                      for j in (js, jm, jl))
                        ws, wm, wl = (float(wg[g][j]) for j in (js, jm, jl))
                        # t = (x_s*(ws/wm) + x_m); t = t*(wm/wl) + x_l;
                        # out = t*wl — all on DVE: single-engine writers
                        # keep the out-DMA at one sem wait, and measured
                        # runs show the DVE-only store path beats routing
                        # the final scale through ACT.
                        nc.vector.scalar_tensor_tensor(
                            ot[:, g], it[:, i0], ws / wm, it[:, i1],
                            mult, add)
                        nc.vector.scalar_tensor_tensor(
                            ot[:, g], ot[:, g], wm / wl, it[:, i2],
                            mult, add)
                        nc.vector.tensor_scalar(
                            ot[:, g], ot[:, g], wl, None, mult)
                    nc.scalar.dma_start(
                        y[s, :, :, off:off + fw].rearrange("g p f -> p g f"),
                        ot[:])
    return nc


def _softmax_rows(logits: np.ndarray) -> np.ndarray:
    z = logits.astype(np.float32)
    z = z - z.max(axis=-1, keepdims=True)
    e = np.exp(z)
    return e / e.sum(axis=-1, keepdims=True)


def kernel(fine_features: np.ndarray, logits: np.ndarray) -> np.ndarray:
    global _LAST_RESULT
    ff = np.asarray(fine_features, dtype=np.float32)
    wg = _softmax_rows(np.asarray(logits, dtype=np.float32))

    # (B, 96, H, W) -> (16 slices, 24, 128, 512); slicing the outer axis
    # keeps each core's shard a contiguous zero-copy view.
    x16 = ff.reshape(SLICES, NUM_FINE, P, PLANE_F)
    in_maps = [
        {"x": x16[SLICES_PER_CORE * k:SLICES_PER_CORE * (k + 1)]}
        for k in range(N_CORES)
    ]

    nc = build_nc(wg)
    res = run_bass_kernel_spmd(nc, in_maps, core_ids=list(range(N_CORES)))
    _LAST_RESULT = res

    out16 = np.concatenate([res.results[k]["y"] for k in range(N_CORES)],
                           axis=0)  # (16, 8, 128, 512)
    return out16.reshape(B, NCOARSE * NOUT, H, W)
